# revision 19
# baseline (speedup 1.0000x reference)
"""VQ codebook soft-assignment kernel for 8 TRN2 NeuronCores.

Computes q = normalize_rows((1 + ||z_n - c_k||^2)^-1) for z [200000, 256],
c [256, 256] (Student-t / ALPHA=1 cluster assignment).

Strategy (data-parallel over N, centers replicated):
  host:   shard z over 8 cores (pad each shard to 25088 rows), cast to bf16,
          precompute row norms ||z||^2 (exact fp32, split hi/lo bf16) and
          center norms 1+||c||^2 as extra GEMM contraction rows so PSUM
          directly accumulates t = 1 + ||z-c||^2.
  device: DMA-transpose loads z^T (bf16, batched), 3 bf16 matmuls per
          128-row subtile into a shared [128,512] PSUM bank (2 subtiles),
          DVE reciprocal per pair -> r, row sums split ACT/DVE, GPSIMD
          scales r by 1/s, DMA out fp32 in partition-major layout
          (host restores row-major).
"""

import sys

for _p in ("/opt/trn_rl_repo", "/opt/pypackages"):
    if _p not in sys.path:
        sys.path.insert(0, _p)

import ml_dtypes
import numpy as np

import bass_rust
import concourse.bass as bass
import concourse.mybir as mybir
from concourse.bass_utils import run_bass_kernel_spmd
from concourse.tile import TileContext
from concourse.vector_clock import ScopedClock

BF16 = ml_dtypes.bfloat16

N, K, D = 200000, 256, 256
NCORES = 8
NSH = N // NCORES            # 25000 rows per core
MACRO = 512                  # rows per macro (4 x 128-row subtiles)
MACROS = 49                  # macros per core
NPAD = MACRO * MACROS        # 25088 padded rows per core
SUBS = MACRO // 128          # 4 subtiles per macro
TILES = NPAD // 128          # 196 subtiles per core
TBATCH = 4                   # macros per transpose-load and output DMA group
WBLK = NPAD // 14            # zrows block width (multiple of 128)

# engine routing knobs
DVE_ROWSUM_SLOT = 0          # subtile j == slot -> row-sum on DVE, others ACT
OUT_SP_EVERY = 4             # every k-th output DMA dispatched from SP (HWDGE),
                             # the rest from GPSIMD SWDGE queues

_DRAIN_PATCHED = False


def _patch_tile_drain():
    """This walrus build only allows ONE semaphore wait per Drain; Tile's
    kernel-tail drain aggregates every proc's sem onto one instruction.
    Split the waits across a chain of single-wait Drains."""
    global _DRAIN_PATCHED
    if _DRAIN_PATCHED:
        return
    _DRAIN_PATCHED = True

    def _drain_and_barrier(self, tick_clock, wait_clock):
        drain_inst = self.nc.sync.drain()
        wait_clock.add_sem_waits(
            drain_inst.ins, ScopedClock({None: tick_clock.global_clock})
        )
        si = drain_inst.ins.sync_info
        waits = list(si.on_wait)
        if len(waits) > 1:
            si.on_wait = waits[:1]
            for i in range(1, len(waits)):
                extra = self.nc.sync.drain()
                extra.ins.sync_info = mybir.SyncInfo(
                    on_wait=waits[i : i + 1], on_update=[]
                )

        self.nc.all_engine_barrier()
        assert self.sems is not None
        popped = self.nc._tile_sem_poison_stack.pop()
        assert popped is self._sem_poison
        self.nc.clear_and_free_semaphores(list(self.sems.allocated().values()))
        self.nc.all_engine_barrier()

    TileContext._drain_and_barrier = _drain_and_barrier


def _split_excess_waits(nc: bass.Bass, cap: int = 1) -> int:
    """This walrus build allows only `cap` semaphore waits per instruction.
    Hoist surplus waits onto same-engine NoOps inserted just before the
    owning instruction (sequencers are in-order, so blocking semantics are
    preserved)."""
    n_split = 0
    nop_id = [0]
    for fn in nc.m.functions:
        for b in fn.blocks:
            out = []
            changed = False
            for ins in b.instructions:
                si = ins.sync_info
                if si is not None and si.on_wait and len(si.on_wait) > cap:
                    waits = list(si.on_wait)
                    surplus, keep = waits[:-cap], waits[-cap:]
                    for i in range(0, len(surplus), cap):
                        nop = bass_rust.InstNoOp(
                            name=f"I-waitnop-{nop_id[0]}",
                            opcode="NoOp",
                            engine=ins.engine,
                            ins=[],
                            outs=[],
                        )
                        nop_id[0] += 1
                        nop.sync_info = mybir.SyncInfo(
                            on_wait=surplus[i : i + cap], on_update=[]
                        )
                        out.append(nop)
                    si.on_wait = keep
                    n_split += 1
                    changed = True
                out.append(ins)
            if changed:
                b.instructions = out
    return n_split


def build_kernel() -> bass.Bass:
    _patch_tile_drain()
    nc = bass.Bass()
    bf = mybir.dt.bfloat16
    f32 = mybir.dt.float32

    z = nc.dram_tensor("z", [NPAD, D], bf, kind="ExternalInput")
    # zrows packed [68, 8960]: block b (columns b*1792:(b+1)*1792 of the
    # logical [4, 25088]) lives at partition base 32*(b%3), column segment
    # b//3. Matmul lhsT operands must start at partition 0/32/64, and loads
    # price by per-partition bytes, so this spreads the data over 12 useful
    # partitions while keeping every lhsT slice base-aligned.
    zrows = nc.dram_tensor("zrows", [68, 5 * WBLK], bf, kind="ExternalInput")
    ct2 = nc.dram_tensor("ct2", [D, K], bf, kind="ExternalInput")
    # crows replicated at partition bases 0/32/64 to match zrows slices
    crows = nc.dram_tensor("crows", [68, K], bf, kind="ExternalInput")
    # partition-major output: q2[p, tile, k] holds row tile*128+p
    q2 = nc.dram_tensor("q2", [128, TILES, K], f32, kind="ExternalOutput")

    with TileContext(nc) as tc:
        with (
            tc.tile_pool(name="const", bufs=1) as constp,
            tc.tile_pool(name="zt", bufs=4) as ztp,
            tc.tile_pool(name="r", bufs=10) as rp,
            tc.tile_pool(name="qo", bufs=3) as qp,
            tc.tile_pool(name="s", bufs=8) as sp_,
            tc.tile_pool(name="junk", bufs=2) as jp,
            tc.tile_pool(name="psum", bufs=6, space="PSUM") as pp,
        ):
            # ct2 [256, 256] -> [128, 2, 256]: chunk c holds rows d = c*128+p
            ct2_sb = constp.tile([128, 2, K], bf)
            nc.sync.dma_start(
                out=ct2_sb[:], in_=ct2.rearrange("(c p) k -> p c k", p=128)
            )
            crows_sb = constp.tile([68, K], bf)
            nc.sync.dma_start(out=crows_sb[:], in_=crows[:])
            zrows_sb = constp.tile([68, 5 * WBLK], bf)
            nc.sync.dma_start(out=zrows_sb[:], in_=zrows[:])

            for g0 in range(0, MACROS, TBATCH):
                gn = min(TBATCH, MACROS - g0)
                rows = gn * MACRO
                n0 = g0 * MACRO
                # z^T via DMA transpose: [128 d, rows n] per 128-d chunk
                zt = ztp.tile([128, 2, TBATCH * MACRO], bf, tag="zt")
                for c in range(2):
                    nc.sync.dma_start(
                        out=zt[:, c, :rows],
                        in_=z[n0 : n0 + rows, c * 128 : (c + 1) * 128],
                        transpose=True,
                    )

                qt = qp.tile([128, TBATCH * SUBS, K], f32, tag="qt")
                for mi in range(gn):
                    m = g0 + mi
                    s = sp_.tile([128, SUBS], f32, tag="s")
                    rs = sp_.tile([128, SUBS], f32, tag="rs")
                    rpair = []
                    for half in range(2):
                        pt = pp.tile([128, 2, K], f32, tag="pt")
                        for jj in range(2):
                            j = half * 2 + jj
                            na = m * MACRO + j * 128
                            zo = mi * MACRO + j * 128
                            nc.tensor.matmul(
                                pt[:, jj, :],
                                lhsT=zt[:, 0, zo : zo + 128],
                                rhs=ct2_sb[:, 0, :],
                                start=True,
                                stop=False,
                            )
                            nc.tensor.matmul(
                                pt[:, jj, :],
                                lhsT=zt[:, 1, zo : zo + 128],
                                rhs=ct2_sb[:, 1, :],
                                start=False,
                                stop=False,
                            )
                            # norm rows: += zsq_hi + zsq_lo + (1+csq)_hi + _lo
                            zb, zoff = divmod(na, WBLK)
                            zbase = 32 * (zb % 3)
                            zcol = (zb // 3) * WBLK + zoff
                            nc.tensor.matmul(
                                pt[:, jj, :],
                                lhsT=zrows_sb[
                                    zbase : zbase + 4, zcol : zcol + 128
                                ],
                                rhs=crows_sb[zbase : zbase + 4, :],
                                start=False,
                                stop=True,
                            )
                        r = rp.tile([128, 2, K], f32, tag="r")
                        nc.vector.reciprocal(r[:], pt[:])
                        for jj in range(2):
                            j = half * 2 + jj
                            if j == DVE_ROWSUM_SLOT:
                                nc.vector.reduce_sum(
                                    s[:, j : j + 1],
                                    r[:, jj, :],
                                    axis=mybir.AxisListType.X,
                                )
                            else:
                                junk = jp.tile([128, K], bf, tag="junk")
                                nc.scalar.activation(
                                    junk[:],
                                    r[:, jj, :],
                                    mybir.ActivationFunctionType.Copy,
                                    accum_out=s[:, j : j + 1],
                                )
                        rpair.append(r)

                    nc.vector.reciprocal(rs[:], s[:])
                    for j in range(SUBS):
                        nc.gpsimd.tensor_scalar_mul(
                            qt[:, mi * SUBS + j, :],
                            rpair[j // 2][:, j % 2, :],
                            rs[:, j : j + 1],
                        )
                nc.sync.dma_start(
                    out=q2[:, g0 * SUBS : (g0 + gn) * SUBS, :],
                    in_=qt[:, : gn * SUBS, :],
                )
    n = _split_excess_waits(nc)
    print(f"[kernel] split waits on {n} instructions", file=sys.stderr)
    return nc


def _split_hi_lo(x32: np.ndarray) -> tuple[np.ndarray, np.ndarray]:
    hi = x32.astype(BF16)
    lo = (x32 - hi.astype(np.float32)).astype(BF16)
    return hi, lo


def prepare_inputs(z: np.ndarray, cluster_centers: np.ndarray):
    z = np.ascontiguousarray(z, dtype=np.float32)
    c = np.ascontiguousarray(cluster_centers, dtype=np.float32)

    ct2 = np.ascontiguousarray((-2.0 * c.T).astype(BF16))          # [D, K]
    csq = 1.0 + np.einsum("kd,kd->k", c, c, dtype=np.float32)
    csq_hi, csq_lo = _split_hi_lo(csq)
    crows = np.zeros((68, K), dtype=BF16)
    for base in (0, 32, 64):
        crows[base + 0] = BF16(1.0)
        crows[base + 1] = BF16(1.0)
        crows[base + 2] = csq_hi
        crows[base + 3] = csq_lo

    zsq = np.einsum("nd,nd->n", z, z, dtype=np.float32)
    z_bf = z.astype(BF16)

    in_maps = []
    for i in range(NCORES):
        lo_, hi_ = i * NSH, (i + 1) * NSH
        zs = np.zeros((NPAD, D), dtype=BF16)
        zs[:NSH] = z_bf[lo_:hi_]
        zr = np.zeros((4, NPAD), dtype=BF16)
        h, l = _split_hi_lo(zsq[lo_:hi_])
        zr[0, :NSH] = h
        zr[1, :NSH] = l
        zr[2] = BF16(1.0)
        zr[3] = BF16(1.0)
        wb = NPAD // 14
        zr2 = np.zeros((68, 5 * wb), dtype=BF16)
        for b in range(14):
            zr2[32 * (b % 3) : 32 * (b % 3) + 4, (b // 3) * wb : (b // 3 + 1) * wb] = (
                zr[:, b * wb : (b + 1) * wb]
            )
        in_maps.append({"z": zs, "zrows": zr2, "ct2": ct2, "crows": crows})
    return in_maps


_CACHED = {}


def kernel(z: np.ndarray, cluster_centers: np.ndarray, _trace=False, _tmpdir=None):
    in_maps = prepare_inputs(z, cluster_centers)
    nc = _CACHED.get("nc")
    if nc is None:
        nc = _CACHED["nc"] = build_kernel()
    kw = {}
    if _trace:
        kw = dict(trace=True, tmpdir=_tmpdir)
    res = run_bass_kernel_spmd(nc, in_maps, core_ids=list(range(NCORES)), **kw)
    out = np.empty((N, K), dtype=np.float32)
    for i in range(NCORES):
        q2 = res.results[i]["q2"]  # [128, TILES, K], row = tile*128 + p
        qi = np.swapaxes(q2, 0, 1).reshape(NPAD, K)
        out[i * NSH : (i + 1) * NSH] = qi[:NSH]
    if _trace:
        return out, res
    return out
